# revision 1
# baseline (speedup 1.0000x reference)
"""AngleEnsemble TRN2 kernel: von Mises mean-shift via Jacobi-Anger moments.

Math: softmax mixture w = (1/3) sum_m softmax(logits_m). Mean-shift iterates
theta <- atan2(S(theta), C(theta)) with C,S = sum_n w_n exp(kappa cos(theta-theta_n)) {cos,sin}theta_n.
Expanding exp(kappa cos phi) = I0 + 2 sum_k Ik cos(k phi) (truncated at K), C and S
become trig polynomials in theta whose per-batch coefficients are linear in w:
one fp16 matmul exp(logits) @ F' [360, NCOL] produces [Z | coeffC | coeffS]
(Z = softmax normalizer via the ones column; a global 1/64 scale keeps fp16 in
range and cancels in the final normalize). Iterations then need only
cos/sin(k theta), generated from the unit vector z=(cos,sin) by complex doubling,
and atan2+cos+sin collapse into a Newton-rsqrt normalize — no transcendentals,
no ACT table switches. Phase 2 runs as two 32-column all-DVE chains; the first
is interleaved with phase 1's second half, and each half's head MLP runs as
soon as that half's iterations finish.
"""
import numpy as np
from contextlib import ExitStack

import concourse.bass as bass
import concourse.bacc as bacc
import concourse.mybir as mybir
from concourse.tile import TileContext
from concourse.bass_utils import run_bass_kernel_spmd

F32 = mybir.dt.float32
F16 = mybir.dt.float16
I32 = mybir.dt.int32
AF = mybir.ActivationFunctionType
OP = mybir.AluOpType

M, B, N = 3, 65536, 360
NCORES = 8
BS = B // NCORES          # 8192 batch rows per core
KORD = 10                 # Jacobi-Anger truncation order
NPC = 2 * KORD + 1        # 21 real power columns [c_0..c_K | s_1..s_K]
PBLK = NPC + 1            # 22 = padded block (zero pad col for fp16 alignment)
NCOL = 1 + 2 * PBLK       # 45 = Z | coeffC+pad | coeffS+pad
NK = 120                  # n-chunk (3 chunks of 120 = 360)
SUP = 2048                # b superchunk for DMA/exp staging
NSUP = BS // SUP          # 4
NJ = BS // 128            # 64 column-groups of 128 b
HALF = NJ // 2            # phase-2 processed in two j-halves of 32
GRP = 8                   # b128-chunks per PSUM tile in phase 1
MS_ITERS = 10
MAGIC = 0x5F3759DF


def build(nc: bass.Bass):
    lg = nc.declare_dram_parameter("logitsT", [M, N, BS], F32, isOutput=False)
    sv = nc.declare_dram_parameter("sin_vecT", [3, BS], F16, isOutput=False)
    fp = nc.declare_dram_parameter("Fp", [3, NK, NCOL], F16, isOutput=False)
    w1b = nc.declare_dram_parameter("W1b", [5, 128], F16, isOutput=False)
    w2 = nc.declare_dram_parameter("W2", [128, 2], F16, isOutput=False)
    b2r = nc.declare_dram_parameter("b2r", [128, 2], F32, isOutput=False)
    eye = nc.declare_dram_parameter("eye", [128, 128], F16, isOutput=False)
    out = nc.declare_dram_parameter("out", [BS, 2], F32, isOutput=True)

    with TileContext(nc) as tc, ExitStack() as ctx:
        consts = ctx.enter_context(tc.tile_pool(name="consts", bufs=1))
        state = ctx.enter_context(tc.tile_pool(name="state", bufs=1))
        stage = ctx.enter_context(tc.tile_pool(name="stage", bufs=4))
        epool = ctx.enter_context(tc.tile_pool(name="epool", bufs=2))
        ph1 = ctx.enter_context(tc.tile_pool(name="ph1", bufs=3))
        headp = ctx.enter_context(tc.tile_pool(name="headp", bufs=3))
        psum = ctx.enter_context(tc.tile_pool(name="psum", bufs=4, space="PSUM"))
        psumh = ctx.enter_context(tc.tile_pool(name="psumh", bufs=2, space="PSUM"))

        # ---- constants ----
        fp_t = consts.tile([NK, 3, NCOL], F16)
        for k in range(3):
            nc.sync.dma_start(out=fp_t[:, k, :], in_=fp[k])
        w1b_t = consts.tile([5, 128], F16)
        nc.sync.dma_start(out=w1b_t[:], in_=w1b[:, :])
        w2_t = consts.tile([128, 2], F16)
        nc.sync.dma_start(out=w2_t[:], in_=w2[:, :])
        b2r_t = consts.tile([128, 2], F32)
        nc.sync.dma_start(out=b2r_t[:], in_=b2r[:, :])
        eye_t = consts.tile([128, 128], F16)
        nc.sync.dma_start(out=eye_t[:], in_=eye[:, :])

        # ---- persistent per-b coefficient stash (fp16): b = 128*j + p ----
        coeffC = state.tile([128, NJ, PBLK], F16)
        coeffS = state.tile([128, NJ, PBLK], F16)

        # ================= phase 1 unit: one (superchunk, m) =================
        def phase1_block(s, m):
            b0 = s * SUP
            es = []
            for k in range(3):
                lg_t = stage.tile([NK, SUP], F32, name=f"lg_{s}_{m}_{k}", tag="lg")
                nc.sync.dma_start(
                    out=lg_t[:], in_=lg[m, k * NK:(k + 1) * NK, b0:b0 + SUP]
                )
                e_t = epool.tile([NK, SUP], F16, name=f"e_{s}_{m}_{k}", tag=f"e{k}")
                nc.scalar.activation(out=e_t[:], in_=lg_t[:], func=AF.Exp)
                es.append(e_t)
            for g in range(SUP // (128 * GRP)):
                ps = psum.tile([128, GRP, NCOL], F32, name=f"mom_{s}_{m}_{g}", tag="mom")
                for j in range(GRP):
                    cols = (g * GRP + j) * 128
                    for k in range(3):
                        nc.tensor.matmul(
                            ps[:, j, :], es[k][:, cols:cols + 128], fp_t[:, k, :],
                            start=(k == 0), stop=(k == 2),
                        )
                rz = ph1.tile([128, GRP], F32, name=f"rz_{s}_{m}_{g}", tag="rz")
                nc.vector.reciprocal(rz[:], ps[:, :, 0])
                # fp16 copy of the moments at 1/64 scale (fp16 range; the global
                # scale cancels in the final normalize)
                mom = ph1.tile([128, GRP, NCOL], F16, name=f"mo_{s}_{m}_{g}", tag="mo")
                nc.scalar.activation(out=mom[:], in_=ps[:], func=AF.Copy, scale=1.0 / 64.0)
                jc = s * (SUP // 128) + g * GRP
                bc = rz[:, :, None].broadcast_to([128, GRP, PBLK])
                csl = coeffC[:, jc:jc + GRP, :]
                ssl = coeffS[:, jc:jc + GRP, :]
                if m == 0:
                    nc.gpsimd.tensor_tensor(csl, mom[:, :, 1:1 + PBLK], bc, OP.mult)
                    nc.gpsimd.tensor_tensor(ssl, mom[:, :, 1 + PBLK:NCOL], bc, OP.mult)
                else:
                    tmc = ph1.tile([128, GRP, PBLK], F16, name=f"tmc_{s}_{m}_{g}", tag="tmc")
                    tms = ph1.tile([128, GRP, PBLK], F16, name=f"tms_{s}_{m}_{g}", tag="tms")
                    nc.gpsimd.tensor_tensor(tmc[:], mom[:, :, 1:1 + PBLK], bc, OP.mult)
                    nc.gpsimd.tensor_tensor(csl, csl, tmc[:], OP.add)
                    nc.gpsimd.tensor_tensor(tms[:], mom[:, :, 1 + PBLK:NCOL], bc, OP.mult)
                    nc.gpsimd.tensor_tensor(ssl, ssl, tms[:], OP.add)

        # ================= phase 2: all-DVE chains over j-halves =================
        CI, SI = 1, 1 + KORD  # base col of c_1 / s_1 in P

        class Chain:
            def __init__(self, jlo, w, label):
                eng = nc.vector
                self.eng, self.jlo, self.w = eng, jlo, w
                self.P = state.tile([128, w, PBLK], F16, name=f"P_{label}")
                self.cC = coeffC[:, jlo:jlo + w, :]
                self.cS = coeffS[:, jlo:jlo + w, :]
                self.CSred = state.tile([128, w, 2], F32, name=f"CSred_{label}")
                self.Cred = self.CSred[:, :, 0]
                self.Sred = self.CSred[:, :, 1]
                self.sq2 = state.tile([128, w, 2], F32, name=f"sq2_{label}")
                self.prodC = state.tile([128, w, PBLK], F16, name=f"prodC_{label}")
                self.prodS = state.tile([128, w, PBLK], F16, name=f"prodS_{label}")
                self.r2 = state.tile([128, w], F32, name=f"r2_{label}")
                self.t2 = state.tile([128, w], F32, name=f"t2_{label}")
                self.yn = state.tile([128, w], F32, name=f"yn_{label}")
                self.ta = state.tile([128, w, KORD // 2], F16, name=f"ta_{label}")
                self.tb = state.tile([128, w, KORD // 2], F16, name=f"tb_{label}")

            def init(self):
                eng, P = self.eng, self.P
                eng.memset(P[:, :, 0], 1.0)
                eng.memset(P[:, :, NPC], 0.0)  # pad col
                eng.tensor_copy(self.Cred, self.cC[:, :, 0])
                eng.tensor_copy(self.Sred, self.cS[:, :, 0])
                self.normalize()

            def normalize(self):
                eng, P = self.eng, self.P
                r2, t2, yn = self.r2, self.t2, self.yn
                eng.tensor_tensor(self.sq2[:], self.CSred[:], self.CSred[:], OP.mult)
                eng.tensor_tensor(r2[:], self.sq2[:, :, 0], self.sq2[:, :, 1], OP.add)
                r2i, yi = r2.bitcast(I32), yn.bitcast(I32)
                eng.tensor_scalar(yi, r2i, 1, None, OP.logical_shift_right)
                eng.tensor_scalar(yi, yi, -1, MAGIC, OP.mult, OP.add)
                eng.tensor_tensor(t2[:], yn[:], yn[:], OP.mult)
                eng.tensor_tensor(t2[:], t2[:], r2[:], OP.mult)
                eng.tensor_scalar(t2[:], t2[:], -0.5, 1.5, OP.mult, OP.add)
                eng.tensor_tensor(yn[:], yn[:], t2[:], OP.mult)
                eng.tensor_tensor(
                    P[:, :, CI:SI + 1:SI - CI], self.CSred[:],
                    yn[:, :, None].broadcast_to([128, self.w, 2]), OP.mult)

            def iter_once(self):
                eng, P, w = self.eng, self.P, self.w
                ta, tb = self.ta, self.tb
                mlen = 1
                while mlen < KORD:
                    ww = min(mlen, KORD - mlen)
                    cm = P[:, :, CI + mlen - 1:CI + mlen].broadcast_to([128, w, ww])
                    sm = P[:, :, SI + mlen - 1:SI + mlen].broadcast_to([128, w, ww])
                    cj = P[:, :, CI:CI + ww]
                    sj = P[:, :, SI:SI + ww]
                    eng.tensor_tensor(ta[:, :, :ww], cm, cj, OP.mult)
                    eng.tensor_tensor(tb[:, :, :ww], sm, sj, OP.mult)
                    eng.tensor_tensor(P[:, :, CI + mlen:CI + mlen + ww],
                                      ta[:, :, :ww], tb[:, :, :ww], OP.subtract)
                    eng.tensor_tensor(ta[:, :, :ww], sm, cj, OP.mult)
                    eng.tensor_tensor(tb[:, :, :ww], cm, sj, OP.mult)
                    eng.tensor_tensor(P[:, :, SI + mlen:SI + mlen + ww],
                                      ta[:, :, :ww], tb[:, :, :ww], OP.add)
                    mlen += ww
                eng.tensor_tensor(self.prodC[:], self.cC, P[:], OP.mult)
                eng.tensor_reduce(self.Cred, self.prodC[:, :, 0:NPC],
                                  mybir.AxisListType.X, OP.add)
                eng.tensor_tensor(self.prodS[:], self.cS, P[:], OP.mult)
                eng.tensor_reduce(self.Sred, self.prodS[:, :, 0:NPC],
                                  mybir.AxisListType.X, OP.add)
                self.normalize()

        # ================= head MLP (per half) =================
        fusedT = state.tile([5, BS], F16)
        nc.sync.dma_start(out=fusedT[0:3, :], in_=sv[:, :])
        out_all = state.tile([128, NJ, 2], F32)
        sq = state.tile([128, NJ, 2], F32)
        r2o = state.tile([128, NJ], F32)
        yo = state.tile([128, NJ], F32)
        to = state.tile([128, NJ], F32)

        def head_range(jlo, w, cd):
            # reshape [128, w] c/s columns to [1, w*128] rows of fusedT via a
            # PE transpose (PSUM) + fp16 copy + two contiguous-burst DMAs
            tpin = headp.tile([128, 128], F16, name=f"tpin_{jlo}", tag="tpin")
            nc.vector.tensor_copy(tpin[:, 0:w], cd.P[:, :, CI])
            nc.vector.tensor_copy(tpin[:, w:2 * w], cd.P[:, :, SI])
            pst = psumh.tile([128, 2, 128], F16, name=f"pst_{jlo}", tag="h")
            nc.tensor.transpose(pst[:, 0, :], tpin[:], eye_t[:])
            csfT = headp.tile([128, 128], F16, name=f"csfT_{jlo}", tag="csfT")
            nc.scalar.activation(out=csfT[:], in_=pst[:, 0, :], func=AF.Copy)
            nc.sync.dma_start(
                out=fusedT[3:4, jlo * 128:(jlo + w) * 128].rearrange(
                    "r (j p) -> r j p", p=128),
                in_=csfT[0:w, :],
            )
            nc.sync.dma_start(
                out=fusedT[4:5, jlo * 128:(jlo + w) * 128].rearrange(
                    "r (j p) -> r j p", p=128),
                in_=csfT[w:2 * w, :],
            )
            for jb in range(jlo // 16, (jlo + w) // 16):   # blocks of 16 j
                ps2 = psumh.tile([128, 16, 2], F32, name=f"o_{jb}", tag="o")
                for jj in range(0, 16, 4):                    # 4 j per relu batch
                    j = jb * 16 + jj
                    ps1 = psumh.tile([128, 4, 128], F32, name=f"h_{j}", tag="h")
                    for u in range(4):
                        nc.tensor.matmul(
                            ps1[:, u, :], w1b_t[:],
                            fusedT[:, (j + u) * 128:(j + u + 1) * 128],
                            start=True, stop=True,
                        )
                    hT = headp.tile([128, 4, 128], F16, name=f"hT_{j}", tag="hT")
                    nc.scalar.activation(out=hT[:], in_=ps1[:], func=AF.Relu)
                    for u in range(4):
                        nc.tensor.matmul(ps2[:, jj + u, :], hT[:, u, :], w2_t[:],
                                         start=True, stop=True)
                bcb = b2r_t[:, None, :].broadcast_to([128, 16, 2])
                nc.vector.tensor_tensor(out_all[:, jb * 16:(jb + 1) * 16, :],
                                        ps2[:], bcb, OP.add)
            # row-normalize this range: out /= max(|out|, 1e-12)
            oa = out_all[:, jlo:jlo + w, :]
            sqh = sq[:, jlo:jlo + w, :]
            r2h = r2o[:, jlo:jlo + w]
            yh = yo[:, jlo:jlo + w]
            th = to[:, jlo:jlo + w]
            nc.vector.tensor_tensor(sqh, oa, oa, OP.mult)
            nc.vector.tensor_tensor(r2h, sqh[:, :, 0], sqh[:, :, 1], OP.add)
            r2i, yi = r2h.bitcast(I32), yh.bitcast(I32)
            nc.vector.tensor_scalar(yi, r2i, 1, None, OP.logical_shift_right)
            nc.vector.tensor_scalar(yi, yi, -1, MAGIC, OP.mult, OP.add)
            for _ in range(3):
                nc.vector.tensor_tensor(th, yh, yh, OP.mult)
                nc.vector.tensor_tensor(th, th, r2h, OP.mult)
                nc.vector.tensor_scalar(th, th, -0.5, 1.5, OP.mult, OP.add)
                nc.vector.tensor_tensor(yh, yh, th, OP.mult)
            nc.vector.tensor_scalar(yh, yh, 1e12, None, OP.min)
            nc.vector.tensor_tensor(
                oa, oa, yh[:, :, None].broadcast_to([128, w, 2]), OP.mult)
            nc.sync.dma_start(
                out=out[jlo * 128:(jlo + w) * 128].rearrange(
                    "(j p) c -> p j c", p=128),
                in_=oa,
            )

        # ================= schedule =================
        # two 32-wide phase-2 halves: half 0 interleaves with phase 1's
        # second half; half 1 pairs with half 0's tail.
        for s in (0, 1):
            for m in range(M):
                phase1_block(s, m)
        c0 = Chain(0, HALF, "h0")
        c0.init()
        d0 = 0
        for s in (2, 3):
            for m in range(M):
                phase1_block(s, m)
                if d0 < MS_ITERS:
                    c0.iter_once(); d0 += 1
        c1 = Chain(HALF, HALF, "h1")
        c1.init()
        d1 = 0
        while d0 < MS_ITERS:
            c0.iter_once(); d0 += 1
            if d1 < MS_ITERS:
                c1.iter_once(); d1 += 1
        head_range(0, HALF, c0)
        while d1 < MS_ITERS:
            c1.iter_once(); d1 += 1
        head_range(HALF, HALF, c1)

def _build_Fp():
    """F' [3, NK, NCOL] fp16: exp-logits -> [Z | coeffC+pad | coeffS+pad]."""
    # I_k(10) for k=0..KORD+1, hardcoded (scipy.special.iv(k, 10.0))
    iv10 = [
        2815.716628466254, 2670.988303701255, 2281.518967726004,
        1758.380716166120, 1226.490565693291, 777.1882064830589,
        449.3022898718774, 238.0255847757819, 116.0661461102767,
        52.31922632375539, 21.89170616206518, 8.536924495442690,
        3.119276255343020, 1.071597692949700,
    ]
    K = KORD
    n = np.arange(N)
    th = 2 * np.pi * n / N
    c = np.array([iv10[0]] + [2 * iv10[k] for k in range(1, K + 2)])
    A = np.cos(np.outer(np.arange(K + 2), th))   # [K+2, N]
    Bm = np.sin(np.outer(np.arange(K + 2), th))
    cols = [np.ones(N)]
    cols.append(c[0] * A[1])                          # CA_0
    for k in range(1, K + 1):
        cols.append(c[k] / 2 * (A[k - 1] + A[k + 1]))  # CA_k
    for k in range(1, K + 1):
        cols.append(c[k] / 2 * (Bm[k - 1] + Bm[k + 1]))  # CB_k
    cols.append(np.zeros(N))                          # pad
    cols.append(c[0] * Bm[1])                         # SB_0
    for k in range(1, K + 1):
        cols.append(c[k] / 2 * (Bm[k + 1] - Bm[k - 1]))  # SB_k
    for k in range(1, K + 1):
        cols.append(c[k] / 2 * (A[k - 1] - A[k + 1]))  # SA_k
    cols.append(np.zeros(N))                          # pad
    Fp = np.stack(cols, axis=1).astype(np.float16)    # [N, NCOL]
    assert Fp.shape[1] == NCOL
    return np.ascontiguousarray(Fp.reshape(3, NK, NCOL))


_NC_CACHE = {}


def _get_nc():
    if "nc" not in _NC_CACHE:
        nc = bacc.Bacc("TRN2", target_bir_lowering=False, debug=False,
                       enable_asserts=True, num_devices=NCORES)
        build(nc)
        nc.compile()
        _NC_CACHE["nc"] = nc
    return _NC_CACHE["nc"]


def kernel(von_logits, sin_vec, W1, b1, W2, b2, _trace=False, _trace_kwargs=None):
    vT = np.ascontiguousarray(
        np.asarray(von_logits, np.float32).transpose(0, 2, 1)
    )  # [M, N, B]
    svT = np.concatenate([
        np.asarray(sin_vec, np.float32).T,
        np.ones((1, B), np.float32),
    ], axis=0).astype(np.float16)  # [3, B] rows: sv0, sv1, ones
    Fp = _build_Fp()
    W1f = np.asarray(W1, np.float32)
    W1b = np.ascontiguousarray(np.concatenate(
        [W1f[0:2], np.asarray(b1, np.float32)[None, :], W1f[2:4]], 0
    ).astype(np.float16))
    W2f = np.ascontiguousarray(np.asarray(W2, np.float32).astype(np.float16))
    b2rep = np.ascontiguousarray(np.broadcast_to(np.asarray(b2, np.float32), (128, 2)))
    eye16 = np.ascontiguousarray(np.eye(128, dtype=np.float16))

    in_maps = []
    for ci in range(NCORES):
        sl = slice(ci * BS, (ci + 1) * BS)
        in_maps.append({
            "logitsT": np.ascontiguousarray(vT[:, :, sl]),
            "sin_vecT": np.ascontiguousarray(svT[:, sl]),
            "Fp": Fp, "W1b": W1b, "W2": W2f, "b2r": b2rep, "eye": eye16,
        })

    nc = _get_nc()
    kw = {}
    if _trace:
        kw = {"trace": True, "trace_kwargs": _trace_kwargs or {}}
    res = run_bass_kernel_spmd(nc, in_maps, core_ids=list(range(NCORES)), **kw)
    outs = [r["out"] for r in res.results]
    full = np.concatenate(outs, axis=0).astype(np.float32)
    if _trace:
        kernel._last_results = res
    return full



# revision 53
# speedup vs baseline: 1.4538x; 1.4538x over previous
"""AngleEnsemble TRN2 kernel: von Mises mean-shift via Jacobi-Anger moments.

Math: softmax mixture w = (1/3) sum_m softmax(logits_m). Mean-shift iterates
theta <- atan2(S(theta), C(theta)) with C,S = sum_n w_n exp(kappa cos(theta-theta_n)) {cos,sin}theta_n.
Expanding exp(kappa cos phi) = I0 + 2 sum_k Ik cos(k phi) (truncated at K=8),
C and S become trig polynomials in theta whose per-batch coefficients are
linear in w: one fp16 matmul exp(logits) @ F' [360, NCOL] produces
[Z | coeffC | coeffS] (Z = softmax normalizer via the ones column; a global
1/64 scale keeps fp16 in range and cancels in the final normalize).

Key layout/perf choices vs the earlier version:
- logits are shipped to the device as fp16 (host cast): halves the HBM
  traffic (the DMA roofline) at ~1e-4 logit error, far inside tolerance.
- exp runs as ONE fused ACT op per (superchunk, m) block [120, 3*2048].
- phase-2 state is k-MAJOR [128 part, harmonic, b]: every tensor_tensor in
  the doubling has its innermost dim packed (b), so the DVE 2x fp16 mode
  applies; broadcasts sit on the middle (harmonic) axis.
- the per-m 1/Z scaling + accumulation is fused into scalar_tensor_tensor
  ops straight out of PSUM (C on DVE, S on GPSIMD): no separate fp16
  moment staging pass.
- per-iteration coefficient reduces run on GPSIMD for the chain that
  overlaps phase 1 (keeps DVE on the doubling critical path).
"""
import numpy as np
from contextlib import ExitStack

import concourse.bass as bass
import concourse.bacc as bacc
import concourse.mybir as mybir
from concourse.tile import TileContext
from concourse.bass_utils import run_bass_kernel_spmd

F32 = mybir.dt.float32
F16 = mybir.dt.float16
I32 = mybir.dt.int32
AF = mybir.ActivationFunctionType
OP = mybir.AluOpType
AX = mybir.AxisListType

M, B, N = 3, 65536, 360
NCORES = 8
BS = B // NCORES          # 8192 batch rows per core
KORD = 8                  # Jacobi-Anger truncation order
NPC = 2 * KORD + 1        # 17 real rows [1 | c_1..c_K | s_1..s_K]
PBLK = NPC + 1            # 18 = padded block (zero pad row)
NCOL = 1 + 2 * PBLK       # 37 = Z | coeffC+pad | coeffS+pad
NK = 120                  # n-chunk (3 chunks of 120 = 360)
SUP = 2048                # b superchunk for DMA/exp staging
NSUP = BS // SUP          # 4
JS = SUP // 128           # 16 j-columns per superchunk
NJ = BS // 128            # 64 column-groups of 128 b
HALF = NJ // 2
MS_ITERS = 10
MAGIC = 0x5F3759DF
CI, SI = 1, 1 + KORD      # P row of c_1 / s_1


def build(nc: bass.Bass):
    lg = nc.declare_dram_parameter("logitsT", [M, N, BS], F16, isOutput=False)
    sv = nc.declare_dram_parameter("sin_vecT", [3, BS], F16, isOutput=False)
    fp = nc.declare_dram_parameter("Fp", [3, NK, NCOL], F16, isOutput=False)
    w1b = nc.declare_dram_parameter("W1b", [7, 128], F16, isOutput=False)
    w2 = nc.declare_dram_parameter("W2", [128, 2], F16, isOutput=False)
    b2r = nc.declare_dram_parameter("b2r", [128, 2], F32, isOutput=False)
    eye = nc.declare_dram_parameter("eye", [128, 128], F16, isOutput=False)
    out = nc.declare_dram_parameter("out", [BS, 2], F32, isOutput=True)

    with TileContext(nc) as tc, ExitStack() as ctx:
        consts = ctx.enter_context(tc.tile_pool(name="consts", bufs=1))
        state = ctx.enter_context(tc.tile_pool(name="state", bufs=1))
        epool = ctx.enter_context(tc.tile_pool(name="epool", bufs=3))
        xpool = ctx.enter_context(tc.tile_pool(name="xpool", bufs=3))
        rpool = ctx.enter_context(tc.tile_pool(name="rpool", bufs=3))
        headp = ctx.enter_context(tc.tile_pool(name="headp", bufs=3))
        psum = ctx.enter_context(tc.tile_pool(name="psum", bufs=2, space="PSUM"))
        psumh = ctx.enter_context(tc.tile_pool(name="psumh", bufs=2, space="PSUM"))

        # ---- ACT table prime: pull the Exp table load off the critical path
        prime = consts.tile([1, 2], F16)
        nc.vector.memset(prime[:], 0.0)
        nc.scalar.activation(out=prime[:, 0:1], in_=prime[:, 1:2], func=AF.Exp)

        # ---- constants on the gpsimd (SWDGE) queue: keeps the SP queue
        # free so the first logit loads issue immediately ----
        fp_t = consts.tile([NK, 3, NCOL], F16)
        for k in range(3):
            nc.gpsimd.dma_start(out=fp_t[:, k, :], in_=fp[k])
        w1b_t = consts.tile([7, 128], F16)
        w2_t = consts.tile([128, 2], F16)
        b2r_t = consts.tile([128, 2], F32)
        eye_t = consts.tile([128, 128], F16)

        def load_late_consts():
            nc.gpsimd.dma_start(out=w1b_t[:], in_=w1b[:, :])
            nc.gpsimd.dma_start(out=w2_t[:], in_=w2[:, :])
            nc.gpsimd.dma_start(out=b2r_t[:], in_=b2r[:, :])
            nc.gpsimd.dma_start(out=eye_t[:], in_=eye[:, :])

        # ---- persistent per-b coefficient stash (fp16, k-major) ----
        coeffC = state.tile([128, PBLK, NJ], F16)
        coeffS = state.tile([128, PBLK, NJ], F16)

        # ================= phase 1 unit: one (superchunk, m) =================
        def phase1_block(s, m, defer=False, split_exp=False):
            b0 = s * SUP
            e_in = epool.tile([NK, 3, SUP], F16, name=f"ei_{s}_{m}", tag="ei")
            for k in range(3):
                nc.sync.dma_start(
                    out=e_in[:, k, :], in_=lg[m, k * NK:(k + 1) * NK, b0:b0 + SUP]
                )
            e_t = xpool.tile([NK, 3, SUP], F16, name=f"e_{s}_{m}", tag="e")
            if split_exp:
                # per-k exps let ACT start on the first landed chunk
                for k in range(3):
                    nc.scalar.activation(out=e_t[:, k, :], in_=e_in[:, k, :],
                                         func=AF.Exp)
            else:
                nc.scalar.activation(out=e_t[:], in_=e_in[:], func=AF.Exp)
            ps = psum.tile([128, JS, NCOL], F32, name=f"mom_{s}_{m}", tag="mom")
            for j in range(JS):
                cols = j * 128
                for k in range(3):
                    nc.tensor.matmul(
                        ps[:, j, :], e_t[:, k, cols:cols + 128], fp_t[:, k, :],
                        start=(k == 0), stop=(k == 2),
                    )
            rz = rpool.tile([128, 1, JS], F32, name=f"rz_{s}_{m}", tag="rz")
            jc = s * JS
            csl = coeffC[:, :, jc:jc + JS]
            ssl = coeffS[:, :, jc:jc + JS]
            psC = ps[:, :, 1:1 + PBLK].transpose([0, 2, 1])       # [128,PBLK,JS]
            psS = ps[:, :, 1 + PBLK:NCOL].transpose([0, 2, 1])
            rzb = rz.broadcast_to([128, PBLK, JS])
            # the DVE/Pool accumulate section, optionally deferred so it can
            # be interleaved into chain-iteration stall points
            closures = []

            def c_recip():
                nc.vector.reciprocal(rz[:, 0, :], ps[:, :, 0])
            closures.append(c_recip)
            # GPSIMD cannot touch PSUM (HW restriction), so the accumulates
            # run on DVE, interleaved into chain-iteration stall points.
            eng = nc.vector
            if m == 0:
                def c_acc():
                    eng.scalar_tensor_tensor(
                        ssl, psS, 1.0 / 64.0, rzb, OP.mult, OP.mult)
                    eng.scalar_tensor_tensor(
                        csl, psC, 1.0 / 64.0, rzb, OP.mult, OP.mult)
                closures.append(c_acc)
            else:
                tmc = rpool.tile([128, PBLK, JS], F16, name=f"tmc_{s}_{m}", tag="tmc")
                tms = rpool.tile([128, PBLK, JS], F16, name=f"tms_{s}_{m}", tag="tms")

                def c_acc1():
                    eng.scalar_tensor_tensor(
                        tms[:], psS, 1.0 / 64.0, rzb, OP.mult, OP.mult)
                    eng.scalar_tensor_tensor(
                        tmc[:], psC, 1.0 / 64.0, rzb, OP.mult, OP.mult)

                def c_acc2():
                    eng.tensor_tensor(ssl, ssl, tms[:], OP.add)
                    eng.tensor_tensor(csl, csl, tmc[:], OP.add)
                closures.append(c_acc1)
                closures.append(c_acc2)
            if defer:
                return closures
            for c in closures:
                c()
            return []

        # ================= phase 2: chains over j-ranges (k-major state) ====
        class Chain:
            def __init__(self, jlo, w, label, prod_eng):
                self.jlo, self.w = jlo, w
                self.prod_eng = prod_eng
                self.P = state.tile([128, PBLK, w], F16, name=f"P_{label}")
                self.cC = coeffC[:, :, jlo:jlo + w]
                self.cS = coeffS[:, :, jlo:jlo + w]
                self.CS = state.tile([128, 2, w], F32, name=f"CS_{label}")
                self.prod = state.tile([128, 2, PBLK, w], F16, name=f"pr_{label}")
                self.sq2 = state.tile([128, 2, w], F32, name=f"sq_{label}")
                self.r2 = state.tile([128, 1, w], F32, name=f"r2_{label}")
                self.t2 = state.tile([128, 1, w], F32, name=f"t2_{label}")
                self.yn = state.tile([128, 1, w], F32, name=f"yn_{label}")
                self.ta = state.tile([128, KORD // 2, w], F16, name=f"ta_{label}")
                self.tb = state.tile([128, KORD // 2, w], F16, name=f"tb_{label}")
                self.tc = state.tile([128, KORD // 2, w], F16, name=f"tc_{label}")
                self.td = state.tile([128, KORD // 2, w], F16, name=f"td_{label}")
                self.vz = state.tile([128, 2, w], F32, name=f"vz_{label}")

            def init(self):
                v = nc.vector
                v.memset(self.P[:, 0, :], 1.0)
                v.memset(self.P[:, NPC:PBLK, :], 0.0)
                v.tensor_copy(self.CS[:, 0, :], self.cC[:, 0, :])
                v.tensor_copy(self.CS[:, 1, :], self.cS[:, 0, :])
                self.normalize()

            def _doubling(self):
                # four independent products per stage, both P-row writes at
                # the end: every consumer sits >=2 ops behind its producer so
                # the ~95ns RAW pipeline latency stays hidden.
                v, P, w = nc.vector, self.P, self.w
                mlen = 1
                while mlen < KORD:
                    ww = min(mlen, KORD - mlen)
                    cm = P[:, CI + mlen - 1:CI + mlen, :].broadcast_to([128, ww, w])
                    sm = P[:, SI + mlen - 1:SI + mlen, :].broadcast_to([128, ww, w])
                    cj = P[:, CI:CI + ww, :]
                    sj = P[:, SI:SI + ww, :]
                    ta, tb = self.ta[:, 0:ww, :], self.tb[:, 0:ww, :]
                    tc, td = self.tc[:, 0:ww, :], self.td[:, 0:ww, :]
                    v.tensor_tensor(ta, cm, cj, OP.mult)
                    v.tensor_tensor(tb, sm, sj, OP.mult)
                    v.tensor_tensor(tc, sm, cj, OP.mult)
                    v.tensor_tensor(td, cm, sj, OP.mult)
                    v.tensor_tensor(P[:, CI + mlen:CI + mlen + ww, :], ta, tb,
                                    OP.subtract)
                    v.tensor_tensor(P[:, SI + mlen:SI + mlen + ww, :], tc, td,
                                    OP.add)
                    mlen += ww

            def _prods(self, eng=None):
                pe = eng or self.prod_eng
                pe.tensor_tensor(self.prod[:, 0], self.cC, self.P[:], OP.mult)
                pe.tensor_tensor(self.prod[:, 1], self.cS, self.P[:], OP.mult)

            def _reduce(self):
                prv = self.prod.transpose([0, 1, 3, 2])[:, :, :, 0:NPC]
                nc.vector.tensor_reduce(self.CS[:], prv, AX.X, OP.add)

            def _norm_ops(self, zout=None):
                # magic-rsqrt + 1 Newton step as a closure list (one op each)
                # so solo iterations can slot fillers into the serial chain's
                # stall points and paired iterations can zip two chains.
                # zout: alternate destination for the unit vector (the head's
                # transpose staging buffer on the final iteration).
                v, w = nc.vector, self.w
                r2, t2, yn = self.r2[:, 0, :], self.t2[:, 0, :], self.yn[:, 0, :]
                r2i = self.r2.bitcast(I32)[:, 0, :]
                yi = self.yn.bitcast(I32)[:, 0, :]
                zrows = zout if zout is not None \
                    else self.P[:, CI:SI + 1:SI - CI, :]
                return [
                    lambda: v.tensor_tensor(self.sq2[:], self.CS[:],
                                            self.CS[:], OP.mult),
                    lambda: v.tensor_tensor(r2, self.sq2[:, 0, :],
                                            self.sq2[:, 1, :], OP.add),
                    lambda: v.tensor_scalar(yi, r2i, 1, None,
                                            OP.logical_shift_right),
                    lambda: v.tensor_scalar(yi, yi, -1, MAGIC,
                                            OP.mult, OP.add),
                    lambda: v.tensor_tensor(
                        self.vz[:], self.CS[:],
                        self.yn.broadcast_to([128, 2, w]), OP.mult),
                    lambda: v.tensor_tensor(t2, yn, yn, OP.mult),
                    lambda: v.tensor_tensor(t2, t2, r2, OP.mult),
                    lambda: v.tensor_scalar(t2, t2, -0.5, 1.5,
                                            OP.mult, OP.add),
                    lambda: v.tensor_tensor(
                        zrows, self.vz[:],
                        self.t2.broadcast_to([128, 2, w]), OP.mult),
                ]

            def normalize(self, F=None, zout=None):
                F = F or (lambda: None)
                ops = self._norm_ops(zout)
                for i, op in enumerate(ops):
                    op()
                    if i in (0, 2, 3, 6):
                        F()

            def iter_once(self, fill=(), prod_eng=None, zout=None):
                fill = list(fill)

                def F():
                    if fill:
                        fill.pop(0)()
                self._doubling()
                self._prods(prod_eng)
                F()
                self._reduce()
                self.normalize(F, zout)
                while fill:
                    fill.pop(0)()

        def pair_iter(cA, cB, zoutA=None):
            # one iteration of both chains, op-interleaved: every RAW edge
            # has the buddy chain's independent ops between producer and
            # consumer, so the DVE pipeline never stalls.
            cA._doubling()
            cA._prods()
            cB._doubling()
            cA._reduce()
            cB._prods()
            na, nb = cA._norm_ops(zoutA), cB._norm_ops()
            na[0]()
            na[1]()
            cB._reduce()
            for a, b in zip(na[2:], nb):
                a()
                b()
            for b in nb[len(na) - 2:]:
                b()

        # ================= head MLP (per chain half) =================
        # fusedX rows: 0 = cos, 1 = sin, 2-3 absorb the unused XBAR
        # transpose rows (W1b has zero rows there), 4-6 = sv0, sv1, ones.
        fusedX = state.tile([7, BS], F16)
        nc.vector.memset(fusedX[:], 0.0)
        tpin = state.tile([128, 128], F16)
        nc.vector.memset(tpin[:, 2 * JS:128], 0.0)
        out_all = state.tile([128, NJ, 2], F32)
        sq = state.tile([128, NJ, 2], F32)
        r2o = state.tile([128, NJ], F32)
        yo = state.tile([128, NJ], F32)
        to = state.tile([128, NJ], F32)

        def head_pre(jlo, w, cd, skip_copies=False):
            # c/s rows of P -> one XBAR DMA-transpose into fusedX rows 3,4
            # (rows 5,6 catch the pad columns); then per-128-col MLP matmuls.
            c0_ = jlo - cd.jlo
            if not skip_copies:
                nc.vector.tensor_copy(tpin[:, 0:w], cd.P[:, CI, c0_:c0_ + w])
                nc.vector.tensor_copy(tpin[:, w:2 * w], cd.P[:, SI, c0_:c0_ + w])
            pst = psumh.tile([128, 2, 128], F16, name=f"pst_{jlo}", tag="h")
            nc.tensor.transpose(pst[:, 0, :], tpin[:], eye_t[:])
            csfT = headp.tile([128, 128], F16, name=f"csfT_{jlo}", tag="csfT")
            nc.scalar.activation(out=csfT[:], in_=pst[:, 0, :], func=AF.Copy)
            nc.sync.dma_start(
                out=fusedX[0:1, jlo * 128:(jlo + w) * 128].rearrange(
                    "r (j p) -> r j p", p=128),
                in_=csfT[0:w, :],
            )
            nc.gpsimd.dma_start(
                out=fusedX[1:2, jlo * 128:(jlo + w) * 128].rearrange(
                    "r (j p) -> r j p", p=128),
                in_=csfT[w:2 * w, :],
            )
            ps2s = []
            for jb in range(jlo // 16, (jlo + w) // 16):   # blocks of 16 j
                ps2 = psumh.tile([128, 16, 2], F32, name=f"o_{jb}", tag="o")
                for jj in range(0, 16, 4):                 # 4 j per relu batch
                    j = jb * 16 + jj
                    ps1 = psumh.tile([128, 4, 128], F32, name=f"h_{j}", tag="h")
                    for u in range(4):
                        nc.tensor.matmul(
                            ps1[:, u, :], w1b_t[:],
                            fusedX[0:7, (j + u) * 128:(j + u + 1) * 128],
                            start=True, stop=True,
                        )
                    hT = headp.tile([128, 4, 128], F16, name=f"hT_{j}", tag="hT")
                    nc.scalar.activation(out=hT[:], in_=ps1[:], func=AF.Relu)
                    for u in range(4):
                        nc.tensor.matmul(ps2[:, jj + u, :], hT[:, u, :], w2_t[:],
                                         start=True, stop=True)
                ps2s.append(ps2)
            return ps2s

        def head_post(jlo, w, ps2s, emit=True):
            # bias add + row-normalize (out /= max(|out|, 1e-12)) + store,
            # as one-op closures so it can fill chain-iteration stalls.
            oa = out_all[:, jlo:jlo + w, :]
            sqh = sq[:, jlo:jlo + w, :]
            r2h = r2o[:, jlo:jlo + w]
            yh = yo[:, jlo:jlo + w]
            th = to[:, jlo:jlo + w]
            r2i, yi = r2h.bitcast(I32), yh.bitcast(I32)
            v = nc.vector
            ops = []
            for i, ps2 in enumerate(ps2s):
                jb = jlo // 16 + i
                ops.append(lambda jb=jb, ps2=ps2: v.tensor_tensor(
                    out_all[:, jb * 16:(jb + 1) * 16, :], ps2[:],
                    b2r_t[:, None, :].broadcast_to([128, 16, 2]), OP.add))
            ops.append(lambda: v.tensor_tensor(sqh, oa, oa, OP.mult))
            ops.append(lambda: v.tensor_tensor(r2h, sqh[:, :, 0],
                                               sqh[:, :, 1], OP.add))
            ops.append(lambda: v.tensor_scalar(yi, r2i, 1, None,
                                               OP.logical_shift_right))
            ops.append(lambda: v.tensor_scalar(yi, yi, -1, MAGIC,
                                               OP.mult, OP.add))
            for _ in range(2):
                ops.append(lambda: v.tensor_tensor(th, yh, yh, OP.mult))
                ops.append(lambda: v.tensor_tensor(th, th, r2h, OP.mult))
                ops.append(lambda: v.tensor_scalar(th, th, -0.5, 1.5,
                                                   OP.mult, OP.add))
                ops.append(lambda: v.tensor_tensor(yh, yh, th, OP.mult))
            ops.append(lambda: v.tensor_scalar(yh, yh, 1e12, None, OP.min))
            ops.append(lambda: v.tensor_tensor(
                oa, oa, yh[:, :, None].broadcast_to([128, w, 2]), OP.mult))
            ops.append(lambda: nc.sync.dma_start(
                out=out[jlo * 128:(jlo + w) * 128].rearrange(
                    "(j p) c -> p j c", p=128),
                in_=oa,
            ))
            if emit:
                for op in ops:
                    op()
                return []
            return ops

        # ================= schedule =================
        # c0={s0,s1} w32 iterates while s2/s3 stream in (4 solo iterations
        # with phase-1 fills), then the remaining 6 run op-paired with
        # c1={s2,s3}; c1's solo tail carries the c0-head post ops as fills.
        zv32 = tpin[:, 0:2 * HALF].rearrange("p (r w) -> p r w", w=HALF)
        phase1_block(0, 0, split_exp=True)
        phase1_block(0, 1)
        phase1_block(0, 2)
        load_late_consts()
        nc.gpsimd.dma_start(out=fusedX[4:7, :], in_=sv[:, :])
        for m in range(3):
            phase1_block(1, m)
        c0 = Chain(0, HALF, "h0", prod_eng=nc.vector)
        c0.init()
        d0 = 0
        pend = []
        blocks = [(s, m) for s in (2, 3) for m in range(3)]
        for i, (s, m) in enumerate(blocks):
            if d0 < 4:
                c0.iter_once(fill=pend); d0 += 1
                pend = phase1_block(s, m, defer=True)
            else:
                for op in pend:
                    op()
                pend = phase1_block(s, m, defer=True)
        for op in pend:
            op()
        c1 = Chain(HALF, HALF, "h1", prod_eng=nc.vector)
        c1.init()
        d1 = 0
        while d0 < MS_ITERS:
            pair_iter(c0, c1, zoutA=zv32 if d0 == MS_ITERS - 1 else None)
            d0 += 1; d1 += 1
        ha = head_pre(0, HALF, c0, skip_copies=True)
        fills = head_post(0, HALF, ha, emit=False)
        c1.iter_once(); d1 += 1  # head_pre PE/ACT lands
        while d1 < MS_ITERS:
            c1.iter_once(fill=fills[:6],
                         zout=zv32 if d1 == MS_ITERS - 1 else None)
            fills = fills[6:]; d1 += 1
        for op in fills:
            op()
        hc = head_pre(HALF, HALF, c1, skip_copies=True)
        head_post(HALF, HALF, hc)


def _build_Fp():
    """F' [3, NK, NCOL] fp16: exp-logits -> [Z | coeffC+pad | coeffS+pad]."""
    # I_k(10) for k=0..KORD+1, hardcoded (scipy.special.iv(k, 10.0))
    iv10 = [
        2815.716628466254, 2670.988303701255, 2281.518967726004,
        1758.380716166120, 1226.490565693291, 777.1882064830589,
        449.3022898718774, 238.0255847757819, 116.0661461102767,
        52.31922632375539, 21.89170616206518, 8.536924495442690,
        3.119276255343020, 1.071597692949700,
    ]
    K = KORD
    n = np.arange(N)
    th = 2 * np.pi * n / N
    c = np.array([iv10[0]] + [2 * iv10[k] for k in range(1, K + 2)])
    A = np.cos(np.outer(np.arange(K + 2), th))   # [K+2, N]
    Bm = np.sin(np.outer(np.arange(K + 2), th))
    cols = [np.ones(N)]
    cols.append(c[0] * A[1])                          # CA_0
    for k in range(1, K + 1):
        cols.append(c[k] / 2 * (A[k - 1] + A[k + 1]))  # CA_k
    for k in range(1, K + 1):
        cols.append(c[k] / 2 * (Bm[k - 1] + Bm[k + 1]))  # CB_k
    cols.append(np.zeros(N))                          # pad
    cols.append(c[0] * Bm[1])                         # SB_0
    for k in range(1, K + 1):
        cols.append(c[k] / 2 * (Bm[k + 1] - Bm[k - 1]))  # SB_k
    for k in range(1, K + 1):
        cols.append(c[k] / 2 * (A[k - 1] - A[k + 1]))  # SA_k
    cols.append(np.zeros(N))                          # pad
    Fp = np.stack(cols, axis=1).astype(np.float16)    # [N, NCOL]
    assert Fp.shape[1] == NCOL
    return np.ascontiguousarray(Fp.reshape(3, NK, NCOL))


_NC_CACHE = {}


def _get_nc():
    if "nc" not in _NC_CACHE:
        nc = bacc.Bacc("TRN2", target_bir_lowering=False, debug=False,
                       enable_asserts=True, num_devices=NCORES)
        build(nc)
        nc.compile()
        _NC_CACHE["nc"] = nc
    return _NC_CACHE["nc"]


def kernel(von_logits, sin_vec, W1, b1, W2, b2, _trace=False, _trace_kwargs=None):
    vT = np.ascontiguousarray(
        np.asarray(von_logits, np.float32).transpose(0, 2, 1).astype(np.float16)
    )  # [M, N, B] fp16
    svT = np.concatenate([
        np.asarray(sin_vec, np.float32).T,
        np.ones((1, B), np.float32),
    ], axis=0).astype(np.float16)  # [3, B] rows: sv0, sv1, ones
    Fp = _build_Fp()
    W1f = np.asarray(W1, np.float32)
    zr = np.zeros((2, 128), np.float32)
    W1b = np.ascontiguousarray(np.concatenate(
        [W1f[2:4], zr, W1f[0:2], np.asarray(b1, np.float32)[None, :]], 0
    ).astype(np.float16))
    W2f = np.ascontiguousarray(np.asarray(W2, np.float32).astype(np.float16))
    b2rep = np.ascontiguousarray(np.broadcast_to(np.asarray(b2, np.float32), (128, 2)))
    eye16 = np.ascontiguousarray(np.eye(128, dtype=np.float16))

    in_maps = []
    for ci in range(NCORES):
        sl = slice(ci * BS, (ci + 1) * BS)
        in_maps.append({
            "logitsT": np.ascontiguousarray(vT[:, :, sl]),
            "sin_vecT": np.ascontiguousarray(svT[:, sl]),
            "Fp": Fp, "W1b": W1b, "W2": W2f, "b2r": b2rep, "eye": eye16,
        })

    nc = _get_nc()
    kw = {}
    if _trace:
        kw = {"trace": True, "trace_kwargs": _trace_kwargs or {}}
    res = run_bass_kernel_spmd(nc, in_maps, core_ids=list(range(NCORES)), **kw)
    outs = [r["out"] for r in res.results]
    full = np.concatenate(outs, axis=0).astype(np.float32)
    if _trace:
        kernel._last_results = res
    return full
